# revision 41
# baseline (speedup 1.0000x reference)
"""Causal self-attention (B=2, N=2048, D=1024, H=16, hd=64) on 8 trn2 NeuronCores.

Sharding: core c handles batch b = c//4 and 4 heads hs = [4*(c%4) .. 4*(c%4)+3]
(tensor-parallel over heads x data-parallel over batch). Each core computes its
heads' attention and a row-parallel partial of the output projection; the host
sums the 4 partials per batch and adds the output bias.

v2: mixed-precision fp8/bf16 device algorithm.
  - qk projection: fp8e4m3 DoubleRow matmuls (2x128 contraction per
    instruction) producing qkT8 in a "quad32" layout: head h occupies
    partitions [32h, 32h+32); free dim groups {k-half0, k-half1, q-half0,
    q-half1} so score matmuls contract the 64-dim head via [32, 2] DoubleRow.
  - scores: fp8 DoubleRow, one matmul per (head, key-tile).
  - attn@v: off-diagonal key-tile PAIRS via fp8 DoubleRow over v65_8
    ([128, 2, 65] = v columns + a ones column accumulating the softmax
    denominator); diagonal tiles in bf16 (accurate v for high-weight keys).
  - exp on ScalarE (scale=1/8 folded), causal tri-mask multiplied into the
    bf16 at tiles on DVE (2-byte fast path).
  - rows 0..127 (few-key softmax rows, precision critical) recomputed
    exactly in bf16 by a small "protection" path that overwrites
    saT[:, :, 0:128].
  - output projection row-parallel in fp32r; normalize multiplies run on the
    otherwise-idle GPSIMD engine.
"""

import numpy as np
import ml_dtypes
from contextlib import ExitStack

import concourse.bass as bass
import concourse.tile as tile
from concourse import bacc, mybir
from concourse import bass_utils

F32 = mybir.dt.float32
F32R = mybir.dt.float32r
BF16 = mybir.dt.bfloat16
FP8 = mybir.dt.float8e4
EXP = mybir.ActivationFunctionType.Exp
DR = mybir.MatmulPerfMode.DoubleRow

B, N, D, H, HD = 2, 2048, 1024, 16, 64
N_CORES = 8
LH = 4            # local heads per core
KT = D // 128     # 8 contraction k-tiles
NT = N // 128     # 16 n-tiles
NB = N // 512     # 4 n-blocks / q-blocks
QB = 512

_CACHE: dict = {}

# tuning knobs (A/B experiments)
CFG = {
    "at8_bufs": 14,
    "at16_bufs": 12,
    "oe_bufs": 4,
    "sc_bufs": 2,
    "norm_engine": "dve",    # gpsimd cannot read PSUM
    "tri_engine": "pool",    # at16 mask engine: pool | dve
    "psum_dma": False,       # DMA cannot read PSUM directly
    "protect": True,
    "act_primer": True,
    "skip_attention": False,
    "skip_outproj": False,
    "skip_vproj": False,
    "skip_qkproj": False,
    "skip_av": False,
    "skip_norm": False,
    "skip_protattn": False,
    "skip_protav": False,
    "skip_protnorm": False,
}
import os as _os
for _k in ("skip_attention", "skip_outproj", "skip_vproj", "skip_qkproj", "protect",
           "skip_av", "skip_norm", "skip_protattn", "skip_protav", "skip_protnorm"):
    _v = _os.environ.get("K_" + _k.upper())
    if _v is not None:
        CFG[_k] = _v == "1"


def _emit(nc, tc, ctx, io, repeat=1):
    (xT8, xT16, wqk8, wqk16, wv, bqk8, bqk16, bv, wo, tri16, out) = io

    persist = ctx.enter_context(tc.tile_pool(name="persist", bufs=1))
    sbp = ctx.enter_context(tc.tile_pool(name="work", bufs=1))
    psum = ctx.enter_context(tc.tile_pool(name="psum", bufs=1, space="PSUM"))

    # ---- persistent SBUF tensors ----
    xT8_sb = persist.tile([128, KT, N], FP8)
    xT16_sb = persist.tile([128, KT, N], BF16)
    wqk8_sb = persist.tile([128, KT, 512], FP8)
    wqk16_sb = persist.tile([128, KT, 512], BF16)
    wv_sb = persist.tile([128, KT, 256], BF16)
    wo_sb = persist.tile([128, 2, 1024], F32R)
    bqk8_sb = persist.tile([128, 4], F32)
    bqk16_sb = persist.tile([128, 4], F32)
    bv_sb = persist.tile([1, 256], F32R)
    ones_sb = persist.tile([1, 128], F32R)
    warm_sb = persist.tile([1, 512], F32R)
    tri16_sb = persist.tile([128, 128], BF16)
    # [p, {kh0,kh1,qh0,qh1}, n]; head h on partitions [32h, 32h+32)
    qkT8_sb = persist.tile([128, 4, N], FP8)
    # protection block qkT, bf16. Groups {kp, q-z0, q-z1} per pair: the q
    # copies are zero-padded on the other head's partitions so score matmuls
    # can contract the full 128 partitions (device faults on non-DoubleRow
    # matmuls at 32/64-row tile positions when DoubleRow is also in use).
    qkTp_sb = persist.tile([128, 6, 128], BF16)
    v65_16 = persist.tile([128, NT, LH * 65], BF16)
    v65_8 = persist.tile([128, NT, LH * 72], FP8)  # 72-stride: dual-fp8 needs 16B-aligned pair step
    saT_sb = persist.tile([128, 2, N], F32R)

    # ---- input DMAs, batched (HWDGE dispatch ~625ns each). The fp8 x blocks
    # all come first: they feed the score pipeline that keeps the Act engine
    # (the throughput wall) busy; bf16 x / prot weights can land later ----
    xT16v = xT16.rearrange("(t p) n -> p t n", t=KT)
    xT8v = xT8.rearrange("(t p) n -> p t n", t=KT)
    nc.sync.dma_start(bqk8_sb[:], bqk8.rearrange("t p -> p t"))
    wqk8v = wqk8.rearrange("(t p) c -> p t c", t=KT)
    nc.sync.dma_start(wqk8_sb[:, 0:4, :], wqk8v[:, 0:4, :])
    nc.sync.dma_start(xT8_sb[:, 0:4, 0:QB], xT8v[:, 0:4, 0:QB])
    nc.sync.dma_start(wqk8_sb[:, 4:8, :], wqk8v[:, 4:8, :])
    nc.sync.dma_start(xT8_sb[:, 4:8, 0:QB], xT8v[:, 4:8, 0:QB])
    nc.sync.dma_start(xT8_sb[:, :, QB:2 * QB], xT8v[:, :, QB:2 * QB])
    nc.sync.dma_start(bqk16_sb[:], bqk16.rearrange("t p -> p t"))
    nc.sync.dma_start(bv_sb[:], bv[:])
    nc.sync.dma_start(tri16_sb[:], tri16[:])
    nc.sync.dma_start(wv_sb[:], wv.rearrange("(t p) c -> p t c", t=KT))
    nc.sync.dma_start(xT16_sb[:, :, 0:QB], xT16v[:, :, 0:QB])
    for nb in range(2, NB):
        nc.sync.dma_start(xT8_sb[:, :, nb * QB:(nb + 1) * QB],
                          xT8v[:, :, nb * QB:(nb + 1) * QB])
    nc.sync.dma_start(wqk16_sb[:], wqk16.rearrange("(t p) c -> p t c", t=KT))
    for nb in range(1, NB):
        nc.sync.dma_start(xT16_sb[:, :, nb * QB:(nb + 1) * QB],
                          xT16v[:, :, nb * QB:(nb + 1) * QB])
    nc.sync.dma_start(wo_sb[:], wo.rearrange("(t p) c -> p t c", t=2))
    nc.vector.memset(ones_sb[:].bitcast(F32), 1.0)
    nc.vector.memset(warm_sb[:].bitcast(F32), 1.0)
    for _w in range(CFG.get("warm_mms", 10)):
        wp = psum.tile([128, 512], F32, name="ps_op", tag="op", bufs=2)
        nc.tensor.matmul(wp[0:1, :], warm_sb[0:1, 0:1], warm_sb[0:1, :],
                         start=True, stop=True)
    # only the ones-columns need init; v copies fill the rest
    nc.vector.memset(
        v65_16[:, :, :].rearrange("p t (h c) -> p t h c", c=65)[:, :, :, 64:65], 1.0)
    nc.vector.memset(
        v65_8[:, :, :].rearrange("p t (h c) -> p t h c", c=72)[:, :, :, 64:65], 1.0)
    if CFG.get("act_primer", True):
        # load the exp table set before the first real activation needs it
        primer = sbp.tile([1, 1], F32, name="t_primer", tag="primer", bufs=1)
        nc.scalar.activation(primer[:], ones_sb[0:1, 0:1].bitcast(F32), EXP)

    # ---- phase helpers ----
    def v_proj(nt):
        if CFG["skip_vproj"]:
            return
        ps = psum.tile([128, 512], F32, name="ps_pj", tag="op", bufs=2)[:, 0:256]
        for kt in range(KT):
            nc.tensor.matmul(
                ps[:], xT16_sb[:, kt, nt * 128:(nt + 1) * 128], wv_sb[:, kt, :],
                start=(kt == 0), stop=False,
            )
        nc.tensor.matmul(ps[:], ones_sb[:], bv_sb[:], start=False, stop=True)
        src = ps[:, :].rearrange("p (h c) -> p h c", c=64)
        v16 = v65_16[:, nt, :].rearrange("p (h c) -> p h c", c=65)[:, :, 0:64]
        nc.vector.tensor_copy(v16, src)
        # fp8 copy derived from the bf16 one (gpsimd cannot read PSUM)
        v8e = nc.gpsimd if CFG.get("v8_engine", "pool") == "pool" else nc.vector
        v8e.tensor_copy(
            v65_8[:, nt, :].rearrange("p (h c) -> p h c", c=72)[:, :, 0:64], v16)

    def qk_proj(nb):
        if CFG["skip_qkproj"]:
            return
        pss = []
        for ct in range(4):
            ps = psum.tile([128, 1024], F32, name="ps_qk0", tag="sc", bufs=2)[:, 0:512]
            pss.append(ps)
            for j in range(4):
                nc.tensor.matmul(
                    ps[:], wqk8_sb[:, 2 * j:2 * j + 2, ct * 128:(ct + 1) * 128],
                    xT8_sb[:, 2 * j:2 * j + 2, nb * QB:(nb + 1) * QB],
                    start=(j == 0), stop=(j == 3), perf_mode=DR,
                )
            if ct == 1:
                for c in range(2):
                    nc.vector.tensor_scalar_add(
                        qkT8_sb[:, c, nb * QB:nb * QB + 128], pss[c][:, 0:128],
                        bqk8_sb[:, c:c + 1])
        for ct, a, b in ((2, 128, 512), (3, 128, 512), (0, 128, 512),
                         (1, 128, 512), (2, 0, 128), (3, 0, 128)):
            nc.vector.tensor_scalar_add(
                qkT8_sb[:, ct, nb * QB + a:nb * QB + b], pss[ct][:, a:b],
                bqk8_sb[:, ct:ct + 1])

    def prot_qk():
        if CFG["skip_protattn"]:
            return
        nc.vector.memset(qkTp_sb[:], 0.0)
        for ct in range(4):
            pp = psum.tile([128, 512], F32, name="ps_pj", tag="op", bufs=2)[:, 0:128]
            for kt in range(KT):
                nc.tensor.matmul(
                    pp[:], wqk16_sb[:, kt, ct * 128:(ct + 1) * 128],
                    xT16_sb[:, kt, 0:128],
                    start=(kt == 0), stop=(kt == KT - 1),
                )
            pr, is_q = divmod(ct, 2)
            if not is_q:
                nc.vector.tensor_scalar_add(
                    qkTp_sb[:, 3 * pr, :], pp[:], bqk16_sb[:, ct:ct + 1])
            else:
                # q copies zero-padded per head: head s only on its own 64
                # partitions so the score matmul can contract all 128 rows
                nc.vector.tensor_scalar_add(
                    qkTp_sb[0:64, 3 * pr + 1, :], pp[0:64, :],
                    bqk16_sb[0:64, ct:ct + 1])
                nc.vector.tensor_scalar_add(
                    qkTp_sb[64:128, 3 * pr + 2, :], pp[64:128, :],
                    bqk16_sb[64:128, ct:ct + 1])

    def prot_attn():
        if CFG["skip_protattn"]:
            return
        accp = psum.tile([128, 512], F32, name="ps_acc", tag="acc", bufs=2)
        for p in range(2):
            ps2 = psum.tile([128, 512], F32, name="ps_pj", tag="op", bufs=2)[:, 0:256]
            for s in range(2):
                # full-128-row matmul; the other head's q partitions are zero
                nc.tensor.matmul(
                    ps2[:, s * 128:(s + 1) * 128],
                    qkTp_sb[:, 3 * p, :],
                    qkTp_sb[:, 3 * p + 1 + s, :],
                    start=True, stop=True,
                )
            atp = sbp.tile([128, 256], BF16, name="t_atp", tag="atp", bufs=2)
            nc.scalar.activation(atp[:], ps2[:], EXP, scale=0.125)
            atv = atp[:].rearrange("p (s c) -> p s c", c=128)
            nc.vector.tensor_mul(
                atv, atv, tri16_sb[:, None, :].broadcast_to([128, 2, 128]))
            for s in range(2):
                if CFG["skip_protav"]:
                    break
                h = 2 * p + s
                nc.tensor.matmul(
                    accp[0:65, h * 128:(h + 1) * 128],
                    v65_16[:, 0, h * 65:h * 65 + 65],
                    atp[:, s * 128:(s + 1) * 128],
                    start=True, stop=True, skip_group_check=True,
                )
        for h in range(LH):
            if CFG["skip_protav"] or CFG["skip_protnorm"]:
                break
            rc = sbp.tile([1, 512], F32, name="t_rc", tag="rc", bufs=2)[:, 0:128]
            nc.vector.reciprocal(rc[:], accp[64:65, h * 128:(h + 1) * 128])
            bc = sbp.tile([64, 512], F32, name="t_bc", tag="bc", bufs=2)[:, 0:128]
            nc.gpsimd.partition_broadcast(bc[:], rc[:])
            po = (h % 2) * 64
            nme = nc.gpsimd if CFG["norm_engine"] == "pool" else nc.vector
            nme.tensor_mul(
                saT_sb[po:po + 64, h // 2, 0:128],
                accp[0:64, h * 128:(h + 1) * 128], bc[:])

    ATT = {}  # (J, p) -> dict(at8=[...], at16=[...])

    def _weave(wv):
        if wv:
            u = wv.pop(0)
            u()

    def att_scores(J, p, weave=None, diag_first=False):
        if CFG["skip_attention"] or CFG["skip_vproj"] or CFG["skip_qkproj"]:
            return
        q_lo = 128 if (J == 0 and CFG["protect"]) else 0
        st = {"at8": [], "at16": [], "q_lo": q_lo, "diag_first": diag_first}
        ATT[(J, p)] = st
        if diag_first:
            _diag_scores(J, p, st, weave)
        for m in range(2 * J):
            at8 = sbp.tile([128, 2, 1024], FP8, name="t_at8", tag="at8",
                           bufs=CFG["at8_bufs"])
            st["at8"].append(at8)
            for half in range(2):
                t = 2 * m + half
                sc = psum.tile([128, 1024], F32, name="ps_sc", tag="sc",
                               bufs=CFG["sc_bufs"])
                for s in range(2):
                    h = 2 * p + s
                    nc.tensor.matmul(
                        sc[:, s * 512:(s + 1) * 512],
                        qkT8_sb[32 * h:32 * h + 32, 0:2, t * 128:(t + 1) * 128],
                        qkT8_sb[32 * h:32 * h + 32, 2:4, J * QB:(J + 1) * QB],
                        start=True, stop=True, perf_mode=DR,
                        tile_position=(32 * h, 0),
                    )
                nc.scalar.activation(at8[:, half, :], sc[:], EXP, scale=0.125)
            _weave(weave)
        if not diag_first:
            _diag_scores(J, p, st, weave)

    def _diag_scores(J, p, st, weave):
        q_lo = st["q_lo"]
        for d in range(4):
            t = 4 * J + d
            c0 = max(d * 128, q_lo)
            sc = psum.tile([128, 1024], F32, name="ps_sc", tag="sc",
                           bufs=CFG["sc_bufs"])
            for s in range(2):
                h = 2 * p + s
                nc.tensor.matmul(
                    sc[:, s * 512 + c0:(s + 1) * 512],
                    qkT8_sb[32 * h:32 * h + 32, 0:2, t * 128:(t + 1) * 128],
                    qkT8_sb[32 * h:32 * h + 32, 2:4, J * QB + c0:(J + 1) * QB],
                    start=True, stop=True, perf_mode=DR,
                    tile_position=(32 * h, 0),
                )
            at16 = sbp.tile([128, 2, 512], BF16, name="t_at16", tag="at16",
                            bufs=CFG["at16_bufs"])
            st["at16"].append((at16, c0))
            scv = sc[:, :].rearrange("p (s c) -> p s c", c=512)
            nc.scalar.activation(at16[:, :, c0:512], scv[:, :, c0:512],
                                 EXP, scale=0.125)
            cm = d * 128
            if cm >= q_lo:
                # causal tri-mask on the diagonal 128-block of both heads
                atv = at16[:, :, cm:cm + 128]
                trie = nc.gpsimd if CFG["tri_engine"] == "pool" else nc.vector
                trie.tensor_mul(
                    atv, atv, tri16_sb[:, None, :].broadcast_to([128, 2, 128]))
            _weave(weave)

    def _diag_av(J, p, st, first):
        accv = st["accv"]
        for d in range(4):
            t = 4 * J + d
            at16, c0 = st["at16"][d]
            for s in range(2):
                h = 2 * p + s
                nc.tensor.matmul(
                    accv[s][0:65, c0:512],
                    v65_16[:, t, h * 65:h * 65 + 65],
                    at16[:, s, c0:512],
                    start=(first and d == 0), stop=False,
                    skip_group_check=True,
                )

    def att_av(J, p, weave=None, chunk_tail=None, part=None):
        if CFG["skip_attention"] or CFG["skip_vproj"] or CFG["skip_qkproj"]:
            return
        if CFG["skip_av"]:
            return
        st = ATT[(J, p)]
        q_lo = st["q_lo"]
        diag_first = st.get("diag_first", False)
        if part != "diag":
            acc0 = psum.tile([128, 512], F32, name="ps_acc", tag="acc", bufs=2)
            acc1 = psum.tile([128, 512], F32, name="ps_acc", tag="acc", bufs=2)
            st["accv"] = (acc0, acc1)
            if diag_first:
                _diag_av(J, p, st, first=True)
        accv = st["accv"]
        for m in (range(2 * J) if part != "diag" else []):
            at8 = st["at8"][m]
            for s in range(2):
                h = 2 * p + s
                nc.tensor.matmul(
                    accv[s][0:65, 0:512],
                    v65_8[:, 2 * m:2 * m + 2, h * 72:h * 72 + 65],
                    at8[:, 0:2, s * 512:(s + 1) * 512],
                    start=(m == 0 and J > 0 and not diag_first), stop=False,
                    perf_mode=DR, skip_group_check=True,
                )
            _weave(weave)
        if part == "off":
            return
        if not diag_first:
            _diag_av(J, p, st, first=(J == 0))
        if CFG["skip_norm"]:
            return
        rcs = []
        for s in range(2):
            rc = sbp.tile([1, 512], F32, name="t_rc", tag="rc", bufs=2)[:, 0:512 - q_lo]
            nc.vector.reciprocal(rc[:], accv[s][64:65, q_lo:512])
            rcs.append(rc)
        def norm_cols(a, b):
            for s in range(2):
                h = 2 * p + s
                bc = sbp.tile([64, 512], F32, name="t_bc", tag="bc", bufs=2)[:, 0:b - a]
                nc.gpsimd.partition_broadcast(bc[:], rcs[s][:, a - q_lo:b - q_lo])
                po = (h % 2) * 64
                nme = nc.gpsimd if CFG["norm_engine"] == "pool" else nc.vector
                nme.tensor_mul(
                    saT_sb[po:po + 64, h // 2, J * QB + a:(J * QB) + b],
                    accv[s][0:64, a:b], bc[:])
        if chunk_tail is None:
            norm_cols(q_lo, 512)
        else:
            # last block: normalize per 128-col chunk and ship each out slab
            # as soon as its columns are ready
            for q in range(4):
                norm_cols(q * 128, (q + 1) * 128)
                chunk_tail(q)

    def out_slab(J, nqs, copy_on_act=False):
        if CFG["skip_outproj"] or CFG["skip_attention"] or CFG["skip_vproj"] or CFG["skip_qkproj"]:
            return
        r0 = J * QB + nqs * 128
        oe = sbp.tile([128, 1024], F32, name="t_oe", tag="oe", bufs=CFG["oe_bufs"])
        for dh in range(2):
            op = psum.tile([128, 512], F32, name="ps_op", tag="op", bufs=2)
            for kt2 in range(2):
                nc.tensor.matmul(
                    op[:],
                    saT_sb[:, kt2, r0:r0 + 128],
                    wo_sb[:, kt2, dh * 512:(dh + 1) * 512],
                    start=(kt2 == 0), stop=(kt2 == 1),
                )
            if copy_on_act and dh == 0:
                # tail: Act is past its last exp and otherwise idle; splitting
                # the two copies across Act/DVE lets them overlap
                nc.scalar.copy(oe[:, dh * 512:(dh + 1) * 512], op[:])
            else:
                nc.vector.tensor_copy(oe[:, dh * 512:(dh + 1) * 512], op[:])
            if copy_on_act:
                # ship each half as soon as it is staged: halves the trailing
                # DMA on the critical path
                nc.sync.dma_start(out[r0:r0 + 128, dh * 512:(dh + 1) * 512],
                                  oe[:, dh * 512:(dh + 1) * 512])
        if not copy_on_act:
            nc.sync.dma_start(out[r0:r0 + 128, :], oe[:])

    def qk_proj_ct(nb, ct):
        if CFG["skip_qkproj"]:
            return
        ps = psum.tile([128, 512], F32, name="ps_pj", tag="op", bufs=2)
        for j in range(4):
            nc.tensor.matmul(
                ps[:], wqk8_sb[:, 2 * j:2 * j + 2, ct * 128:(ct + 1) * 128],
                xT8_sb[:, 2 * j:2 * j + 2, nb * QB:(nb + 1) * QB],
                start=(j == 0), stop=(j == 3), perf_mode=DR,
            )
        nc.vector.tensor_scalar_add(
            qkT8_sb[:, ct, nb * QB:(nb + 1) * QB], ps[:], bqk8_sb[:, ct:ct + 1]
        )

    # ---- emission order: scores stream ahead of everything so the Act
    # engine (the wall) is continuously fed; av/proj/out work fills PE ----
    for _rep in range(repeat):
        qk_proj(0)
        att_scores(0, 0)
        att_scores(0, 1)
        for ct in range(4):
            qk_proj_ct(1, ct)
        att_scores(1, 0)
        att_scores(1, 1, weave=[lambda nt=nt: v_proj(nt) for nt in range(4)])
        for ct in range(4):
            qk_proj_ct(2, ct)
        att_scores(2, 0, weave=(
            [lambda nt=nt: v_proj(nt) for nt in range(4, 8)] + [prot_qk]))
        att_av(0, 0)
        att_av(0, 1)
        att_av(1, 0)
        att_av(1, 1)
        att_scores(2, 1, weave=(
            [prot_attn] + [lambda nt=nt: v_proj(nt) for nt in range(8, 12)]))
        for ct in range(4):
            qk_proj_ct(3, ct)
        att_scores(3, 0, weave=(
            [lambda nt=nt: v_proj(nt) for nt in range(12, 16)]
            + [lambda q=q: out_slab(0, q) for q in range(4)]))
        att_av(2, 0)
        att_av(2, 1)
        att_scores(3, 1, weave=(
            [lambda q=q: out_slab(1, q) for q in range(4)]
            + [lambda q=q: out_slab(2, q) for q in range(4)]),
                   diag_first=True)
        att_av(3, 0)
        att_av(3, 1, part="off")
        att_av(3, 1, part="diag",
               chunk_tail=lambda q: out_slab(3, q, copy_on_act=True))


def build(repeat=1):
    nc = bacc.Bacc("TRN2", target_bir_lowering=False, debug=False,
                   num_devices=N_CORES)
    xT8 = nc.dram_tensor("xT8", [D, N], FP8, kind="ExternalInput").ap()
    xT16 = nc.dram_tensor("xT16", [D, N], BF16, kind="ExternalInput").ap()
    wqk8 = nc.dram_tensor("wqk8", [D, 512], FP8, kind="ExternalInput").ap()
    wqk16 = nc.dram_tensor("wqk16", [D, 512], BF16, kind="ExternalInput").ap()
    wv = nc.dram_tensor("wv", [D, 256], BF16, kind="ExternalInput").ap()
    bqk8 = nc.dram_tensor("bqk8", [4, 128], F32, kind="ExternalInput").ap()
    bqk16 = nc.dram_tensor("bqk16", [4, 128], F32, kind="ExternalInput").ap()
    bv = nc.dram_tensor("bv", [1, 256], F32R, kind="ExternalInput").ap()
    wo = nc.dram_tensor("wo", [256, 1024], F32R, kind="ExternalInput").ap()
    tri16 = nc.dram_tensor("tri16", [128, 128], BF16, kind="ExternalInput").ap()
    out = nc.dram_tensor("out", [N, D], F32, kind="ExternalOutput").ap()

    with tile.TileContext(nc) as tc:
        with ExitStack() as ctx:
            _emit(nc, tc, ctx, (xT8, xT16, wqk8, wqk16, wv, bqk8, bqk16, bv,
                                wo, tri16, out), repeat=repeat)
    nc.compile()
    return nc


def make_in_maps(x, Wqkv, bqkv, Wo):
    """Host-side sharding: per-core input dicts."""
    x = np.asarray(x, dtype=np.float32)
    Wqkv = np.asarray(Wqkv, dtype=np.float32)
    bqkv = np.asarray(bqkv, dtype=np.float32)
    Wo = np.asarray(Wo, dtype=np.float32)
    tri16 = np.triu(np.ones((128, 128), np.float32)).astype(ml_dtypes.bfloat16)
    in_maps = []
    for c in range(N_CORES):
        b, g = divmod(c, 4)
        hs = [4 * g + i for i in range(LH)]
        # source chunk order in Wqkv[h] columns: k (0:64), q (64:128), v (128:192)
        # quad32 layout: ct groups {k-half0, k-half1, q-half0, q-half1};
        # within a group, col 32h+j is head hs[h]'s dim j of that half.
        cols8, bias8 = [], []
        for base in (0, 32, 64, 96):  # k0, k1, q0, q1 halves
            blk = np.concatenate(
                [Wqkv[h][:, base:base + 32] for h in hs], axis=1)
            cols8.append(blk)
            bias8.append(np.concatenate([bqkv[h][base:base + 32] for h in hs]))
        wqk8 = np.concatenate(cols8, axis=1)
        bqk8 = np.stack(bias8)
        # pair layout for the bf16 protection path: {kp0, qp0, kp1, qp1}
        cols16, bias16 = [], []
        for p in range(2):
            hA, hB = hs[2 * p], hs[2 * p + 1]
            cols16 += [Wqkv[hA][:, 0:64], Wqkv[hB][:, 0:64]]
            bias16.append(np.concatenate([bqkv[hA][0:64], bqkv[hB][0:64]]))
            cols16 += [Wqkv[hA][:, 64:128], Wqkv[hB][:, 64:128]]
            bias16.append(np.concatenate([bqkv[hA][64:128], bqkv[hB][64:128]]))
        wqk16 = np.concatenate(cols16, axis=1)
        bqk16 = np.stack(bias16)
        xT = np.ascontiguousarray(x[b].T)
        in_maps.append({
            "xT8": xT.astype(ml_dtypes.float8_e4m3),
            "xT16": xT.astype(ml_dtypes.bfloat16),
            "wqk8": np.ascontiguousarray(wqk8).astype(ml_dtypes.float8_e4m3),
            "wqk16": np.ascontiguousarray(wqk16).astype(ml_dtypes.bfloat16),
            "wv": np.ascontiguousarray(
                np.concatenate([Wqkv[h][:, 128:192] for h in hs], axis=1)
            ).astype(ml_dtypes.bfloat16),
            "bqk8": np.ascontiguousarray(bqk8),
            "bqk16": np.ascontiguousarray(bqk16),
            "bv": np.ascontiguousarray(
                np.concatenate([bqkv[h][128:192] for h in hs])[None, :]),
            "wo": np.ascontiguousarray(
                np.concatenate([Wo[h * HD:(h + 1) * HD, :] for h in hs], axis=0)),
            "tri16": tri16,
        })
    return in_maps


def kernel(x, Wqkv, bqkv, Wo, bo):
    if "nc" not in _CACHE:
        _CACHE["nc"] = build()
    nc = _CACHE["nc"]
    in_maps = make_in_maps(x, Wqkv, bqkv, Wo)
    res = bass_utils.run_bass_kernel_spmd(
        nc, in_maps, core_ids=list(range(N_CORES)))
    bo = np.asarray(bo, dtype=np.float32)
    full = np.empty((B, N, D), dtype=np.float32)
    for b in range(B):
        acc = res.results[4 * b]["out"].astype(np.float32).copy()
        for g in range(1, 4):
            acc += res.results[4 * b + g]["out"]
        full[b] = acc + bo[None, :]
    return full
